# revision 1
# baseline (speedup 1.0000x reference)
"""Cross-graph attention (block-diagonal segment-local attention) on 8 trn2 cores.

Strategy: graphs (batch ids) are contiguous segments in the sorted
atom_batch / residue_batch arrays.  Attention is block-diagonal: atoms of
graph b attend only to residues of graph b.  We shard 4 graphs per core,
pad every graph to a fixed (AG atoms, RG residues) slot so all 8 cores run
one identical SPMD program, and compute per-graph attention with no masks:

  - inputs are packed host-side as transposed tiles atom_h^T (128, A_pad),
    residue_h^T (128, R_pad); zero padding makes padded K columns / V rows
    exactly 0.
  - scores are computed transposed,  S^T = K @ Q^T,  so every matmul takes
    naturally-laid-out operands (no on-device transposes anywhere).
  - all matmuls run in float32r (fast fp32 mode, 1 cycle/row at free>=256).
  - exp(S/sqrt(128) + bias) is one ACT instruction per tile; the per-partition
    bias is 0 for real residues and -30000 for padded ones, so padded
    residues contribute exp = 0 downstream (mask costs zero instructions).
  - V is augmented with a ones column; U = expS^T.T @ [V | 1 | pad] then
    yields both the unnormalized context and the softmax denominator.
  - normalization + residual add run host-side: out = atom_h + U[:, :128]/U[:, 128:129].
"""

import sys

if "/opt/trn_rl_repo" not in sys.path:
    sys.path.insert(0, "/opt/trn_rl_repo")

import numpy as np

import concourse.bass as bass
import concourse.tile as tile
from concourse import bacc, mybir
from concourse.bass_utils import run_bass_kernel_spmd

N_CORES = 8
B = 32                      # number of graphs
P = 128                     # partitions
DH = 128                    # feature dims (DA == DR == DH == 128)
VW = 256                    # U-matmul rhs width (>=256 keeps f32r at full rate)
SCALE = 1.0 / np.sqrt(128.0)
NEG_BIAS = -30000.0

_kernel_cache: dict = {}


def _col_chunks(n):
    """Split n columns into matmul chunks of <=512 that never cross a
    512-element PSUM bank boundary (matmul output must stay in one bank)."""
    out, i = [], 0
    while i < n:
        w = min(512, n - i)
        out.append((i, w))
        i += w
    return out


def _build_kernel(AG: int, RG: int, G: int):
    """One SPMD program: G graph slots of (AG atoms, RG residues) per core."""
    A_pad = G * AG
    R_pad = G * RG
    nkg = RG // P               # residue chunks per graph
    nRc = G * nkg               # residue chunks per core
    ntg = AG // P               # atom chunks per graph
    nAc = G * ntg               # atom chunks per core
    f32 = mybir.dt.float32
    f32r = mybir.dt.float32r

    nc = bacc.Bacc("TRN2")
    atomT = nc.dram_tensor("atomT", [P, A_pad], f32r, kind="ExternalInput")
    resT = nc.dram_tensor("resT", [P, R_pad], f32r, kind="ExternalInput")
    wqT = nc.dram_tensor("wqT", [P, DH], f32r, kind="ExternalInput")
    wkT = nc.dram_tensor("wkT", [P, DH], f32r, kind="ExternalInput")
    wvT = nc.dram_tensor("wvT", [P, DH], f32r, kind="ExternalInput")
    bias = nc.dram_tensor("bias", [P, nRc], f32, kind="ExternalInput")
    out = nc.dram_tensor("out", [A_pad, DH + 1], f32, kind="ExternalOutput")

    sg_chunks = _col_chunks(AG)

    with tile.TileContext(nc) as tc:
        with (
            tc.tile_pool(name="singles", bufs=1) as singles,
            tc.tile_pool(name="psum_big", bufs=3, space="PSUM") as ps_big,
            tc.tile_pool(name="psum_small", bufs=2, space="PSUM") as ps_small,
        ):
            # ---- load everything to SBUF ----
            atomT_sb = singles.tile([P, A_pad], f32r)
            resT_sb = singles.tile([P, R_pad], f32r)
            wqT_sb = singles.tile([P, DH], f32r)
            wkT_sb = singles.tile([P, DH], f32r)
            wvT_sb = singles.tile([P, VW], f32r)
            bias_sb = singles.tile([P, nRc], f32)
            nc.sync.dma_start(wqT_sb[:], wqT[:])
            nc.sync.dma_start(wkT_sb[:], wkT[:])
            nc.vector.memset(wvT_sb[:].bitcast(f32), 0.0)
            nc.sync.dma_start(wvT_sb[:, :DH], wvT[:])
            nc.sync.dma_start(bias_sb[:], bias[:])
            # chunked loads so compute can start on the first chunk
            for i in range(0, R_pad, 512):
                w = min(512, R_pad - i)
                nc.sync.dma_start(resT_sb[:, i : i + w], resT[:, i : i + w])
            for i in range(0, A_pad, 512):
                w = min(512, A_pad - i)
                nc.sync.dma_start(atomT_sb[:, i : i + w], atomT[:, i : i + w])

            # V' = [residue_h @ W_v^T | 1 | junk] laid out per residue chunk
            V_sb = singles.tile([P, nRc, VW], f32r)
            nc.vector.memset(V_sb[:].bitcast(f32), 1.0)

            # ---- Q^T = W_q @ atom_h^T, K^T = W_k @ residue_h^T ----
            # psum->sbuf copies alternate DVE/ACT so neither engine gates PE
            def copy_alt(i, dst, src):
                eng = nc.vector if i % 2 == 0 else nc.scalar
                if eng is nc.vector:
                    eng.tensor_copy(dst, src)
                else:
                    eng.copy(dst, src)

            KT_sb = singles.tile([P, R_pad], f32r)
            for n, i in enumerate(range(0, R_pad, 512)):
                w = min(512, R_pad - i)
                pk = ps_big.tile([P, 512], f32, tag="big")
                nc.tensor.matmul(
                    pk[:, :w], wkT_sb[:], resT_sb[:, i : i + w],
                    start=True, stop=True,
                )
                copy_alt(n, KT_sb[:, i : i + w], pk[:, :w])

            QT_sb = singles.tile([P, A_pad], f32r)
            for n, i in enumerate(range(0, A_pad, 512)):
                w = min(512, A_pad - i)
                pq = ps_big.tile([P, 512], f32, tag="big")
                nc.tensor.matmul(
                    pq[:, :w], wqT_sb[:], atomT_sb[:, i : i + w],
                    start=True, stop=True,
                )
                copy_alt(n + 1, QT_sb[:, i : i + w], pq[:, :w])

            # ---- V chunks (rhs padded to VW cols so f32r runs at rate 1) ----
            for k in range(nRc):
                pv = ps_small.tile([P, VW], f32, tag="small")
                nc.tensor.matmul(
                    pv[:], resT_sb[:, k * P : (k + 1) * P], wvT_sb[:],
                    start=True, stop=True,
                )
                copy_alt(k, V_sb[:, k, :DH], pv[:, :DH])

            # ---- per-graph attention ----
            ES_sb = singles.tile([P, nRc, AG], f32r)   # exp(S^T) per residue chunk
            OUT_sb = singles.tile([P, nAc, DH + 1], f32)

            for g in range(G):
                a0 = g * AG
                for k in range(nkg):
                    kg = g * nkg + k
                    r0 = kg * P
                    ps = ps_big.tile([P, 512 * ((AG + 511) // 512)], f32, tag="big")
                    for c, w in sg_chunks:
                        nc.tensor.matmul(
                            ps[:, c : c + w],
                            KT_sb[:, r0 : r0 + P],
                            QT_sb[:, a0 + c : a0 + c + w],
                            start=True, stop=True,
                        )
                    nc.scalar.activation(
                        ES_sb[:, kg, :], ps[:, :AG],
                        mybir.ActivationFunctionType.Exp,
                        bias=bias_sb[:, kg : kg + 1], scale=SCALE,
                    )

                for t in range(ntg):
                    tg = g * ntg + t
                    pu = ps_small.tile([P, VW], f32, tag="small")
                    for k in range(nkg):
                        kg = g * nkg + k
                        nc.tensor.matmul(
                            pu[:],
                            ES_sb[:, kg, t * P : (t + 1) * P],
                            V_sb[:, kg, :],
                            start=(k == 0), stop=(k == nkg - 1),
                        )
                    nc.vector.tensor_copy(OUT_sb[:, tg, :], pu[:, : DH + 1])

                # stream this graph's rows out while later graphs compute
                nc.sync.dma_start(
                    out[g * AG : (g + 1) * AG, :].rearrange(
                        "(t p) f -> p t f", p=P
                    ),
                    OUT_sb[:, g * ntg : (g + 1) * ntg, :],
                )

    nc.compile()
    return nc


def kernel(atom_h, residue_h, atom_batch, residue_batch, W_q, W_k, W_v):
    atom_h = np.asarray(atom_h, dtype=np.float32)
    residue_h = np.asarray(residue_h, dtype=np.float32)
    atom_batch = np.asarray(atom_batch)
    residue_batch = np.asarray(residue_batch)
    W_q = np.asarray(W_q, dtype=np.float32)
    W_k = np.asarray(W_k, dtype=np.float32)
    W_v = np.asarray(W_v, dtype=np.float32)

    A = atom_h.shape[0]
    R = residue_h.shape[0]
    n_b = max(B, int(atom_batch.max()) + 1 if A else B,
              int(residue_batch.max()) + 1 if R else B)

    ac = np.bincount(atom_batch, minlength=n_b)
    rc = np.bincount(residue_batch, minlength=n_b)
    a_off = np.concatenate([[0], np.cumsum(ac)])
    r_off = np.concatenate([[0], np.cumsum(rc)])

    G = (n_b + N_CORES - 1) // N_CORES
    AG = max(P, int(np.ceil(ac.max() / P)) * P)
    RG = max(P, int(np.ceil(rc.max() / P)) * P)
    A_pad, R_pad = G * AG, G * RG
    nkg = RG // P
    nRc = G * nkg

    key = (AG, RG, G)
    if key not in _kernel_cache:
        _kernel_cache[key] = _build_kernel(AG, RG, G)
    nc = _kernel_cache[key]

    wqT = np.ascontiguousarray(W_q.T)
    wkT = np.ascontiguousarray(W_k.T)
    wvT = np.ascontiguousarray(W_v.T)

    in_maps = []
    for c in range(N_CORES):
        atomT_c = np.zeros((P, A_pad), dtype=np.float32)
        resT_c = np.zeros((P, R_pad), dtype=np.float32)
        bias_c = np.zeros((P, nRc), dtype=np.float32)
        for j in range(G):
            g = c * G + j
            if g >= n_b:
                bias_c[:, j * nkg : (j + 1) * nkg] = NEG_BIAS
                continue
            na, nr = int(ac[g]), int(rc[g])
            if na:
                atomT_c[:, j * AG : j * AG + na] = atom_h[a_off[g] : a_off[g] + na].T
            if nr:
                resT_c[:, j * RG : j * RG + nr] = residue_h[r_off[g] : r_off[g] + nr].T
            flat = np.full(RG, NEG_BIAS, dtype=np.float32)
            flat[:nr] = 0.0
            bias_c[:, j * nkg : (j + 1) * nkg] = flat.reshape(nkg, P).T
        in_maps.append({
            "atomT": atomT_c, "resT": resT_c,
            "wqT": wqT, "wkT": wkT, "wvT": wvT,
            "bias": bias_c,
        })

    res = run_bass_kernel_spmd(nc, in_maps, core_ids=list(range(N_CORES)))

    result = atom_h.copy()
    for c in range(N_CORES):
        u = res.results[c]["out"]
        for j in range(G):
            g = c * G + j
            if g >= n_b:
                continue
            na, nr = int(ac[g]), int(rc[g])
            if na == 0 or nr == 0:
                continue
            rows = u[j * AG : j * AG + na]
            result[a_off[g] : a_off[g] + na] += rows[:, :DH] / rows[:, DH : DH + 1]
    return result



# revision 3
# speedup vs baseline: 1.5363x; 1.5363x over previous
"""Cross-graph attention (block-diagonal segment-local attention) on 8 trn2 cores.

Graphs (batch ids) are contiguous segments in the sorted atom_batch /
residue_batch arrays; attention is block-diagonal per graph.  32 graphs are
assigned 4-per-core to 8 cores, one graph per "slot".  Slot shapes are
variable per rank (slot i on every core has the same (atom-chunks,
residue-chunks, atom-width) capacity = max over the 8 graphs assigned to
that rank), so all 8 cores run one identical SPMD program with minimal
padding.

All matmuls run in bf16 (rate 1 cycle/row at ANY moving width on TRN2, vs
f32r needing >=256), with fp32 PSUM accumulation.  Tolerance is 2e-2; bf16
keeps us well inside it.

Per-core program (everything transposed; no on-device transposes anywhere):
  - CT = M' @ R^T per slot, where M' = (W_q^T W_k)/sqrt(DH) is folded
    host-side from the parameters: kills the separate Q projection.
  - V_k = R_k @ W_v^T per residue chunk ([residues, feats] layout).
  - S^T chunk = CT_k^T @ A^T  ->  exp via one ACT instruction per (slot,
    residue-chunk).  No mask bias needed: padded residues have resT = 0 so
    CT col = 0, exp(0) = 1, but V row = 0 (no numerator pollution) and
    valid = 0 (no denominator pollution).
  - U^T = sum_k V_k^T @ ES_k  (feats x atoms, accumulated in PSUM).
  - denom = sum_k ES_k^T @ valid_k : single-column matmuls, ~free.
  - normalization + residual add run host-side:
      out = atom_h + (U^T / denom)^T.
Software pipelining: S(g+1) is emitted before U(g) so the PE never idles
while ACT computes exp; a dummy exp at t=0 hides the activation-table load
under the input DMAs.  GPSIMD cannot touch PSUM, so PSUM->SBUF copies run
on DVE (ACT is saturated by exp).
"""

import sys

if "/opt/trn_rl_repo" not in sys.path:
    sys.path.insert(0, "/opt/trn_rl_repo")

import numpy as np
import ml_dtypes

import concourse.bass as bass
import concourse.tile as tile
from concourse import bacc, mybir
from concourse.bass_utils import run_bass_kernel_spmd

N_CORES = 8
B = 32                      # number of graphs
P = 128                     # partitions
DH = 128                    # feature dims (DA == DR == DH == 128)

_kernel_cache: dict = {}


def _col_chunks(n):
    """Split n columns into matmul chunks of <=512 that never cross a
    512-element PSUM bank boundary."""
    out, i = [], 0
    while i < n:
        w = min(512, n - i)
        out.append((i, w))
        i += w
    return out


def _build_kernel(slot_shapes):
    """One SPMD program: slots with per-rank (atom-chunks, residue-chunks,
    atom-width) capacities given by slot_shapes = ((AC, RC, AW), ...)."""
    G = len(slot_shapes)
    nAc = sum(s[0] for s in slot_shapes)
    nRc = sum(s[1] for s in slot_shapes)
    A_pad = nAc * P
    R_pad = nRc * P
    a_offs = np.concatenate([[0], np.cumsum([s[0] for s in slot_shapes])])
    r_offs = np.concatenate([[0], np.cumsum([s[1] for s in slot_shapes])])
    max_aw = max(s[2] for s in slot_shapes)

    f32 = mybir.dt.float32
    bf16 = mybir.dt.bfloat16

    nc = bacc.Bacc("TRN2")
    atomT = nc.dram_tensor("atomT", [P, A_pad], bf16, kind="ExternalInput")
    resT = nc.dram_tensor("resT", [P, R_pad], bf16, kind="ExternalInput")
    mT = nc.dram_tensor("mT", [P, DH], bf16, kind="ExternalInput")
    wvT = nc.dram_tensor("wvT", [P, DH], bf16, kind="ExternalInput")
    valid = nc.dram_tensor("valid", [P, nRc], bf16, kind="ExternalInput")
    outT = nc.dram_tensor("outT", [P, A_pad], bf16, kind="ExternalOutput")
    den = nc.dram_tensor("den", [P, nAc], f32, kind="ExternalOutput")

    with tile.TileContext(nc) as tc:
        with (
            tc.tile_pool(name="singles", bufs=1) as singles,
            tc.tile_pool(name="ps_s", bufs=2, space="PSUM") as ps_s,
            tc.tile_pool(name="ps_u", bufs=1, space="PSUM") as ps_u,
            tc.tile_pool(name="ps_kv", bufs=1, space="PSUM") as ps_kv,
            tc.tile_pool(name="ps_d", bufs=1, space="PSUM") as ps_d,
        ):
            atomT_sb = singles.tile([P, A_pad], bf16)
            resT_sb = singles.tile([P, R_pad], bf16)
            mT_sb = singles.tile([P, DH], bf16)
            wvT_sb = singles.tile([P, DH], bf16)
            valid_sb = singles.tile([P, nRc], bf16)
            CT_sb = singles.tile([P, R_pad], bf16)
            V_sb = singles.tile([P, nRc, DH], bf16)
            ES_sb = singles.tile([P, nRc, max_aw], bf16)
            OUT_sb = singles.tile([P, A_pad], bf16)
            DEN_sb = singles.tile([P, nAc], f32)
            warm_sb = singles.tile([P, 8], f32)

            # warm up the ACT exp table while DMAs run
            nc.gpsimd.memset(warm_sb[:], 0.0)
            nc.scalar.activation(
                warm_sb[:, 0:1], warm_sb[:, 1:2],
                mybir.ActivationFunctionType.Exp,
            )

            nc.sync.dma_start(mT_sb[:], mT[:])
            nc.sync.dma_start(wvT_sb[:], wvT[:])
            nc.sync.dma_start(valid_sb[:], valid[:])
            for g in range(G):
                r0, r1 = r_offs[g] * P, r_offs[g + 1] * P
                nc.sync.dma_start(resT_sb[:, r0:r1], resT[:, r0:r1])
                a0, a1 = a_offs[g] * P, a_offs[g + 1] * P
                nc.sync.dma_start(atomT_sb[:, a0:a1], atomT[:, a0:a1])

            # persistent denominator accumulator (one full PSUM bank)
            pd = ps_d.tile([P, 512], f32, tag="d")
            nc.vector.memset(pd[:, : max(nAc, 2)], 0.0)

            def emit_ctv(g):
                """CT and V for slot g's residues."""
                ac, rc, aw = slot_shapes[g]
                if rc == 0:
                    return
                r0 = r_offs[g] * P
                rcols = rc * P
                pc = ps_kv.tile([P, 512], f32, tag="kv")
                nc.tensor.matmul(
                    pc[:, :rcols], mT_sb[:], resT_sb[:, r0 : r0 + rcols],
                    start=True, stop=True,
                )
                nc.vector.tensor_copy(CT_sb[:, r0 : r0 + rcols], pc[:, :rcols])
                pv = ps_kv.tile([P, 512], f32, tag="kv")
                for k in range(rc):
                    nc.tensor.matmul(
                        pv[:, k * P : (k + 1) * P],
                        resT_sb[:, r0 + k * P : r0 + (k + 1) * P],
                        wvT_sb[:],
                        start=True, stop=True,
                    )
                nc.vector.tensor_copy(
                    V_sb[:, r_offs[g] : r_offs[g] + rc, :], pv[:, :rcols]
                )

            def emit_scores(g):
                """S^T chunks + exp for slot g."""
                ac, rc, aw = slot_shapes[g]
                if rc == 0 or aw == 0:
                    return
                a0 = a_offs[g] * P
                for k in range(rc):
                    kg = r_offs[g] + k
                    ps = ps_s.tile([P, 1024], f32, tag="s")
                    for c, w in _col_chunks(aw):
                        nc.tensor.matmul(
                            ps[:, c : c + w],
                            CT_sb[:, kg * P : (kg + 1) * P],
                            atomT_sb[:, a0 + c : a0 + c + w],
                            start=True, stop=True,
                        )
                    nc.scalar.activation(
                        ES_sb[:, kg, :aw], ps[:, :aw],
                        mybir.ActivationFunctionType.Exp,
                    )

            def emit_u(g):
                """U^T accumulation + denominator + output copy for slot g."""
                ac, rc, aw = slot_shapes[g]
                if aw == 0 or rc == 0:
                    return
                a0 = a_offs[g] * P
                pu = ps_u.tile([P, 1024], f32, tag="u")
                for c, w in _col_chunks(aw):
                    for k in range(rc):
                        kg = r_offs[g] + k
                        nc.tensor.matmul(
                            pu[:, c : c + w],
                            V_sb[:, kg, :],
                            ES_sb[:, kg, c : c + w],
                            start=(k == 0), stop=(k == rc - 1),
                        )
                # denominator: single-column matmuls, ES chunk stationary
                for t in range(ac):
                    tw = min(P, aw - t * P)
                    if tw <= 0:
                        break
                    tg = a_offs[g] + t
                    for k in range(rc):
                        kg = r_offs[g] + k
                        nc.tensor.matmul(
                            pd[:tw, tg : tg + 1],
                            ES_sb[:, kg, t * P : t * P + tw],
                            valid_sb[:, kg : kg + 1],
                            start=(k == 0), stop=(k == rc - 1),
                            skip_group_check=True,
                        )
                nc.vector.tensor_copy(OUT_sb[:, a0 : a0 + aw], pu[:, :aw])
                nc.sync.dma_start(outT[:, a0 : a0 + aw], OUT_sb[:, a0 : a0 + aw])

            emit_ctv(0)
            emit_scores(0)
            for g in range(G):
                if g + 1 < G:
                    emit_ctv(g + 1)
                    emit_scores(g + 1)
                emit_u(g)

            nc.vector.tensor_copy(DEN_sb[:], pd[:, :nAc])
            nc.sync.dma_start(den[:], DEN_sb[:])

    nc.compile()
    return nc


def kernel(atom_h, residue_h, atom_batch, residue_batch, W_q, W_k, W_v):
    atom_h = np.asarray(atom_h, dtype=np.float32)
    residue_h = np.asarray(residue_h, dtype=np.float32)
    atom_batch = np.asarray(atom_batch)
    residue_batch = np.asarray(residue_batch)
    W_q = np.asarray(W_q, dtype=np.float32)
    W_k = np.asarray(W_k, dtype=np.float32)
    W_v = np.asarray(W_v, dtype=np.float32)
    bf = ml_dtypes.bfloat16

    A = atom_h.shape[0]
    R = residue_h.shape[0]
    n_b = max(B, int(atom_batch.max()) + 1 if A else B,
              int(residue_batch.max()) + 1 if R else B)

    ac = np.bincount(atom_batch, minlength=n_b)
    rc = np.bincount(residue_batch, minlength=n_b)
    a_off = np.concatenate([[0], np.cumsum(ac)])
    r_off = np.concatenate([[0], np.cumsum(rc)])
    a_ch = np.maximum(1, -(-ac // P))          # atom chunks per graph
    r_ch = np.maximum(1, -(-rc // P))          # residue chunks per graph

    G = (n_b + N_CORES - 1) // N_CORES          # slots per core

    # assign graphs to (rank, core): try a few sort keys, keep the cheapest
    best = None
    for key in (r_ch * 10000 + ac, a_ch * 100000 + r_ch * 10 + ac // 100,
                r_ch * a_ch * 10000 + ac):
        order = np.argsort(-key, kind="stable")
        shapes, cost = [], 0
        for i in range(G):
            grp = order[i * N_CORES : (i + 1) * N_CORES]
            AC = int(a_ch[grp].max()) if len(grp) else 1
            RC = int(r_ch[grp].max()) if len(grp) else 1
            AW = int(-(-int(ac[grp].max()) // 8) * 8) if len(grp) else 8
            AW = max(AW, 8)
            shapes.append((AC, RC, AW))
            cost += RC * AW
        if best is None or cost < best[0]:
            best = (cost, tuple(shapes), order)
    _, slot_shapes, order = best

    nAc = sum(s[0] for s in slot_shapes)
    nRc = sum(s[1] for s in slot_shapes)
    A_pad, R_pad = nAc * P, nRc * P
    a_offs = np.concatenate([[0], np.cumsum([s[0] for s in slot_shapes])])
    r_offs = np.concatenate([[0], np.cumsum([s[1] for s in slot_shapes])])

    key = slot_shapes
    if key not in _kernel_cache:
        _kernel_cache[key] = _build_kernel(slot_shapes)
    nc = _kernel_cache[key]

    scale = 1.0 / np.sqrt(np.float32(DH))
    mT = ((W_q.T @ W_k) * scale).T.astype(bf)   # lhsT for CT = M' @ R^T
    mT = np.ascontiguousarray(mT)
    wvT = np.ascontiguousarray(W_v.T).astype(bf)

    in_maps = []
    for c in range(N_CORES):
        atomT_c = np.zeros((P, A_pad), dtype=bf)
        resT_c = np.zeros((P, R_pad), dtype=bf)
        valid_c = np.zeros((P, nRc), dtype=bf)
        for i in range(G):
            idx = i * N_CORES + c
            if idx >= len(order):
                continue
            g = order[idx]
            na, nr = int(ac[g]), int(rc[g])
            a0, r0 = a_offs[i] * P, r_offs[i] * P
            if na:
                atomT_c[:, a0 : a0 + na] = atom_h[a_off[g] : a_off[g] + na].T.astype(bf)
            if nr:
                resT_c[:, r0 : r0 + nr] = (
                    residue_h[r_off[g] : r_off[g] + nr].T.astype(bf)
                )
                flat = np.zeros(slot_shapes[i][1] * P, dtype=bf)
                flat[:nr] = 1.0
                valid_c[:, r_offs[i] : r_offs[i + 1]] = flat.reshape(-1, P).T
        in_maps.append({
            "atomT": atomT_c, "resT": resT_c,
            "mT": mT, "wvT": wvT, "valid": valid_c,
        })

    res = run_bass_kernel_spmd(nc, in_maps, core_ids=list(range(N_CORES)))

    result = atom_h.copy()
    for c in range(N_CORES):
        ut = np.asarray(res.results[c]["outT"], dtype=np.float32)   # [128, A_pad]
        dn = np.asarray(res.results[c]["den"], dtype=np.float32)    # [128, nAc]
        dn_flat = dn.T.reshape(-1)                                   # atom-col major
        for i in range(G):
            idx = i * N_CORES + c
            if idx >= len(order):
                continue
            g = order[idx]
            na, nr = int(ac[g]), int(rc[g])
            if na == 0 or nr == 0:
                continue
            a0 = a_offs[i] * P
            cols = slice(a0, a0 + na)
            d = dn_flat[cols]
            d = np.where(d > 0, d, 1.0)
            result[a_off[g] : a_off[g] + na] += (ut[:, cols] / d[None, :]).T
    return result


# revision 4
# speedup vs baseline: 1.5513x; 1.0097x over previous
"""Cross-graph attention (block-diagonal segment-local attention) on 8 trn2 cores.

Graphs (batch ids) are contiguous segments in the sorted atom_batch /
residue_batch arrays; attention is block-diagonal per graph.  32 graphs are
assigned 4-per-core to 8 cores, one graph per "slot".  Slot shapes are
variable per rank (slot i on every core has the same (atom-chunks,
residue-chunks, atom-width) capacity = max over the 8 graphs assigned to
that rank), so all 8 cores run one identical SPMD program with minimal
padding.

All matmuls run in bf16 (rate 1 cycle/row at ANY moving width on TRN2, vs
f32r needing >=256), with fp32 PSUM accumulation.  Tolerance is 2e-2; bf16
keeps us well inside it.

Per-core program (everything transposed; no on-device transposes anywhere):
  - CT = M' @ R^T per slot, where M' = (W_q^T W_k)/sqrt(DH) is folded
    host-side from the parameters: kills the separate Q projection.
  - V_k = R_k @ W_v^T per residue chunk ([residues, feats] layout).
  - S^T chunk = CT_k^T @ A^T  ->  exp via one ACT instruction per (slot,
    residue-chunk).  No mask bias needed: padded residues have resT = 0 so
    CT col = 0, exp(0) = 1, but V row = 0 (no numerator pollution) and
    valid = 0 (no denominator pollution).
  - U^T = sum_k V_k^T @ ES_k  (feats x atoms, accumulated in PSUM).
  - denom = sum_k ES_k^T @ valid_k : single-column matmuls, ~free.
  - normalization + residual add run host-side:
      out = atom_h + (U^T / denom)^T.

Scheduling: HWDGE charges ~625ns per DMA (serialized), so ALL inputs live
in one packed dram tensor loaded with 4 per-slot DMAs, and the denominator
rides in the tail of the single output tensor (bf16).  S(g+1) is emitted
before U(g) so the PE never idles while ACT computes exp; a dummy exp at
t=0 hides the activation-table load under the input DMAs.  GPSIMD cannot
touch PSUM, so PSUM->SBUF copies run on DVE (ACT is saturated by exp).
"""

import sys

if "/opt/trn_rl_repo" not in sys.path:
    sys.path.insert(0, "/opt/trn_rl_repo")

import numpy as np
import ml_dtypes

import concourse.bass as bass
import concourse.tile as tile
from concourse import bacc, mybir
from concourse.bass_utils import run_bass_kernel_spmd

N_CORES = 8
B = 32                      # number of graphs
P = 128                     # partitions
DH = 128                    # feature dims (DA == DR == DH == 128)

_kernel_cache: dict = {}


def _col_chunks(n):
    """Split n columns into matmul chunks of <=512 that never cross a
    512-element PSUM bank boundary."""
    out, i = [], 0
    while i < n:
        w = min(512, n - i)
        out.append((i, w))
        i += w
    return out


def _layout(slot_shapes):
    """Packed input/output column layouts shared by builder and packer."""
    G = len(slot_shapes)
    o_res, o_atom, o_out = [], [], []
    nRc = sum(s[1] for s in slot_shapes)
    cur = 2 * DH + nRc                      # [mT | wvT | valid | slots...]
    for ac, rc, aw in slot_shapes:
        o_res.append(cur)
        cur += rc * P
        o_atom.append(cur)
        cur += aw
    W_in = cur
    cur = 0
    for ac, rc, aw in slot_shapes:
        o_out.append(cur)
        cur += aw
    A_out = cur
    nAc = sum(s[0] for s in slot_shapes)
    W_out = A_out + nAc
    return o_res, o_atom, o_out, W_in, A_out, W_out


def _build_kernel(slot_shapes):
    """One SPMD program: slots with per-rank (atom-chunks, residue-chunks,
    atom-width) capacities given by slot_shapes = ((AC, RC, AW), ...)."""
    G = len(slot_shapes)
    nAc = sum(s[0] for s in slot_shapes)
    nRc = sum(s[1] for s in slot_shapes)
    r_offs = np.concatenate([[0], np.cumsum([s[1] for s in slot_shapes])])
    a_offs = np.concatenate([[0], np.cumsum([s[0] for s in slot_shapes])])
    max_aw = max(s[2] for s in slot_shapes)
    o_res, o_atom, o_out, W_in, A_out, W_out = _layout(slot_shapes)
    o_valid = 2 * DH

    f32 = mybir.dt.float32
    bf16 = mybir.dt.bfloat16

    nc = bacc.Bacc("TRN2")
    inp = nc.dram_tensor("inp", [P, W_in], bf16, kind="ExternalInput")
    outp = nc.dram_tensor("outp", [P, W_out], bf16, kind="ExternalOutput")

    with tile.TileContext(nc) as tc:
        with (
            tc.tile_pool(name="singles", bufs=1) as singles,
            tc.tile_pool(name="ps_s", bufs=2, space="PSUM") as ps_s,
            tc.tile_pool(name="ps_u", bufs=1, space="PSUM") as ps_u,
            tc.tile_pool(name="ps_kv", bufs=1, space="PSUM") as ps_kv,
            tc.tile_pool(name="ps_d", bufs=1, space="PSUM") as ps_d,
        ):
            IN_sb = singles.tile([P, W_in], bf16)
            CT_sb = singles.tile([P, nRc * P], bf16)
            V_sb = singles.tile([P, nRc, DH], bf16)
            ES_sb = singles.tile([P, nRc, max_aw], bf16)
            OUT_sb = singles.tile([P, W_out], bf16)
            warm_sb = singles.tile([P, 8], f32)

            # warm up the ACT exp table while DMAs run
            nc.gpsimd.memset(warm_sb[:], 0.0)
            nc.scalar.activation(
                warm_sb[:, 0:1], warm_sb[:, 1:2],
                mybir.ActivationFunctionType.Exp,
            )

            # 4 input DMAs: [aux | slot0], then one per remaining slot
            nc.sync.dma_start(
                IN_sb[:, : o_atom[0] + slot_shapes[0][2]],
                inp[:, : o_atom[0] + slot_shapes[0][2]],
            )
            for g in range(1, G):
                lo, hi = o_res[g], o_atom[g] + slot_shapes[g][2]
                nc.sync.dma_start(IN_sb[:, lo:hi], inp[:, lo:hi])

            mT_ap = IN_sb[:, :DH]
            wvT_ap = IN_sb[:, DH : 2 * DH]

            # persistent denominator accumulator (one full PSUM bank)
            pd = ps_d.tile([P, 512], f32, tag="d")
            nc.vector.memset(pd[:, : max(nAc, 2)], 0.0)

            def emit_ctv(g):
                """CT and V for slot g's residues."""
                ac, rc, aw = slot_shapes[g]
                if rc == 0:
                    return
                r0 = r_offs[g] * P            # CT/V-space offset
                ri = o_res[g]                 # IN_sb offset
                rcols = rc * P
                pc = ps_kv.tile([P, 512], f32, tag="kv")
                nc.tensor.matmul(
                    pc[:, :rcols], mT_ap, IN_sb[:, ri : ri + rcols],
                    start=True, stop=True,
                )
                ct_eng = nc.scalar if g == 0 else nc.vector
                if g == 0:
                    nc.scalar.copy(CT_sb[:, r0 : r0 + rcols], pc[:, :rcols])
                else:
                    nc.vector.tensor_copy(CT_sb[:, r0 : r0 + rcols], pc[:, :rcols])
                pv = ps_kv.tile([P, 512], f32, tag="kv")
                for k in range(rc):
                    nc.tensor.matmul(
                        pv[:, k * P : (k + 1) * P],
                        IN_sb[:, ri + k * P : ri + (k + 1) * P],
                        wvT_ap,
                        start=True, stop=True,
                    )
                nc.vector.tensor_copy(
                    V_sb[:, r_offs[g] : r_offs[g] + rc, :], pv[:, :rcols]
                )

            def emit_scores(g):
                """S^T chunks + exp for slot g."""
                ac, rc, aw = slot_shapes[g]
                if rc == 0 or aw == 0:
                    return
                ai = o_atom[g]
                for k in range(rc):
                    kg = r_offs[g] + k
                    ps = ps_s.tile([P, 1024], f32, tag="s")
                    for c, w in _col_chunks(aw):
                        nc.tensor.matmul(
                            ps[:, c : c + w],
                            CT_sb[:, kg * P : (kg + 1) * P],
                            IN_sb[:, ai + c : ai + c + w],
                            start=True, stop=True,
                        )
                    nc.scalar.activation(
                        ES_sb[:, kg, :aw], ps[:, :aw],
                        mybir.ActivationFunctionType.Exp,
                    )

            def emit_u(g):
                """U^T accumulation + denominator + output copy for slot g."""
                ac, rc, aw = slot_shapes[g]
                if aw == 0 or rc == 0:
                    return
                pu = ps_u.tile([P, 1024], f32, tag="u")
                for c, w in _col_chunks(aw):
                    for k in range(rc):
                        kg = r_offs[g] + k
                        nc.tensor.matmul(
                            pu[:, c : c + w],
                            V_sb[:, kg, :],
                            ES_sb[:, kg, c : c + w],
                            start=(k == 0), stop=(k == rc - 1),
                        )
                # denominator: single-column matmuls, ES chunk stationary
                for t in range(ac):
                    tw = min(P, aw - t * P)
                    if tw <= 0:
                        break
                    tg = a_offs[g] + t
                    for k in range(rc):
                        kg = r_offs[g] + k
                        nc.tensor.matmul(
                            pd[:tw, tg : tg + 1],
                            ES_sb[:, kg, t * P : t * P + tw],
                            IN_sb[:, o_valid + kg : o_valid + kg + 1],
                            start=(k == 0), stop=(k == rc - 1),
                            skip_group_check=True,
                        )
                oo = o_out[g]
                if g == G - 1:
                    # tail: split the copy DVE/ACT and fold in the denominator
                    h = (aw // 2 + 7) // 8 * 8
                    nc.vector.tensor_copy(OUT_sb[:, oo : oo + h], pu[:, :h])
                    nc.scalar.copy(OUT_sb[:, oo + h : oo + aw], pu[:, h:aw])
                    nc.vector.tensor_copy(
                        OUT_sb[:, A_out : A_out + nAc], pd[:, :nAc]
                    )
                    nc.sync.dma_start(outp[:, oo:W_out], OUT_sb[:, oo:W_out])
                else:
                    nc.vector.tensor_copy(OUT_sb[:, oo : oo + aw], pu[:, :aw])
                    nc.sync.dma_start(
                        outp[:, oo : oo + aw], OUT_sb[:, oo : oo + aw]
                    )

            emit_ctv(0)
            emit_scores(0)
            for g in range(G):
                if g + 1 < G:
                    emit_ctv(g + 1)
                    emit_scores(g + 1)
                emit_u(g)

    nc.compile()
    return nc


def kernel(atom_h, residue_h, atom_batch, residue_batch, W_q, W_k, W_v):
    atom_h = np.asarray(atom_h, dtype=np.float32)
    residue_h = np.asarray(residue_h, dtype=np.float32)
    atom_batch = np.asarray(atom_batch)
    residue_batch = np.asarray(residue_batch)
    W_q = np.asarray(W_q, dtype=np.float32)
    W_k = np.asarray(W_k, dtype=np.float32)
    W_v = np.asarray(W_v, dtype=np.float32)
    bf = ml_dtypes.bfloat16

    A = atom_h.shape[0]
    R = residue_h.shape[0]
    n_b = max(B, int(atom_batch.max()) + 1 if A else B,
              int(residue_batch.max()) + 1 if R else B)

    ac = np.bincount(atom_batch, minlength=n_b)
    rc = np.bincount(residue_batch, minlength=n_b)
    a_off = np.concatenate([[0], np.cumsum(ac)])
    r_off = np.concatenate([[0], np.cumsum(rc)])
    a_ch = np.maximum(1, -(-ac // P))          # atom chunks per graph
    r_ch = np.maximum(1, -(-rc // P))          # residue chunks per graph

    G = (n_b + N_CORES - 1) // N_CORES          # slots per core

    # assign graphs to (rank, core): try a few sort keys, keep the cheapest
    best = None
    for key in (r_ch * 10000 + ac, a_ch * 100000 + r_ch * 10 + ac // 100,
                r_ch * a_ch * 10000 + ac):
        order = np.argsort(-key, kind="stable")
        shapes, cost = [], 0
        for i in range(G):
            grp = order[i * N_CORES : (i + 1) * N_CORES]
            AC = int(a_ch[grp].max()) if len(grp) else 1
            RC = int(r_ch[grp].max()) if len(grp) else 1
            AW = int(-(-int(ac[grp].max()) // 8) * 8) if len(grp) else 8
            AW = max(AW, 8)
            shapes.append((AC, RC, AW))
            cost += RC * AW
        if best is None or cost < best[0]:
            best = (cost, tuple(shapes), order)
    _, slot_shapes, order = best

    nAc = sum(s[0] for s in slot_shapes)
    nRc = sum(s[1] for s in slot_shapes)
    a_offs = np.concatenate([[0], np.cumsum([s[0] for s in slot_shapes])])
    r_offs = np.concatenate([[0], np.cumsum([s[1] for s in slot_shapes])])
    o_res, o_atom, o_out, W_in, A_out, W_out = _layout(slot_shapes)
    o_valid = 2 * DH

    key = slot_shapes
    if key not in _kernel_cache:
        _kernel_cache[key] = _build_kernel(slot_shapes)
    nc = _kernel_cache[key]

    scale = 1.0 / np.sqrt(np.float32(DH))
    mT = ((W_q.T @ W_k) * scale).T.astype(bf)   # lhsT for CT = M' @ R^T
    wvT = np.ascontiguousarray(W_v.T).astype(bf)

    in_maps = []
    for c in range(N_CORES):
        inp_c = np.zeros((P, W_in), dtype=bf)
        inp_c[:, :DH] = mT
        inp_c[:, DH : 2 * DH] = wvT
        for i in range(G):
            idx = i * N_CORES + c
            if idx >= len(order):
                continue
            g = order[idx]
            na, nr = int(ac[g]), int(rc[g])
            if na:
                inp_c[:, o_atom[i] : o_atom[i] + na] = (
                    atom_h[a_off[g] : a_off[g] + na].T.astype(bf)
                )
            if nr:
                inp_c[:, o_res[i] : o_res[i] + nr] = (
                    residue_h[r_off[g] : r_off[g] + nr].T.astype(bf)
                )
                flat = np.zeros(slot_shapes[i][1] * P, dtype=bf)
                flat[:nr] = 1.0
                inp_c[:, o_valid + r_offs[i] : o_valid + r_offs[i + 1]] = (
                    flat.reshape(-1, P).T
                )
        in_maps.append({"inp": inp_c})

    res = run_bass_kernel_spmd(nc, in_maps, core_ids=list(range(N_CORES)))

    result = atom_h.copy()
    for c in range(N_CORES):
        full = np.asarray(res.results[c]["outp"], dtype=np.float32)  # [128, W_out]
        dn_flat = full[:, A_out:].T.reshape(-1)                       # atom-col major
        for i in range(G):
            idx = i * N_CORES + c
            if idx >= len(order):
                continue
            g = order[idx]
            na, nr = int(ac[g]), int(rc[g])
            if na == 0 or nr == 0:
                continue
            cols = slice(o_out[i], o_out[i] + na)
            d = dn_flat[a_offs[i] * P : a_offs[i] * P + na]
            d = np.where(d > 0, d, 1.0)
            result[a_off[g] : a_off[g] + na] += (full[:, cols] / d[None, :]).T
    return result
